# revision 26
# baseline (speedup 1.0000x reference)
"""Trainium2 Bass kernel for nn_LossWithBeliveMaps.

loss = mean((prediction - belive_map)^2) where belive_map is 100 Gaussian
(9x9, sigma=2) stamps per image, scattered at integer keypoint coordinates.

v4 design: never materialize the dense believe map.  Expand the MSE:

    sum((p - bm)^2) = sum(p^2) - 2*sum(p*bm) + sum(bm^2)

  * sum(p^2): square+row-accumulate each prediction tile as it lands,
    split between ScalarE (activation accum_out) and DVE (bn_stats; the
    host converts mean/var back to a sum of squares).
  * The Gaussian is separable/rank-1: u(d) = exp(-d^2/8), so with
    yfT[r, k] = u(r - y_k) (r on partitions) a K-accumulating TensorE
    matmul computes s[k, c] = sum_r yfT[r, k] * p[r, c] and
        sum(p*bm) = sum_{k,c} s[k,c] * w_k * u(c - x_k)
    where the dedup weight w_k rides in the xf row factors (exp bias).
  * sum(bm^2) uses the Gaussian-sum identity
    sum_r u(r-y)u(r-y') = sqrt(4 pi) exp(-(y-y')^2/16) (exact for interior
    keypoints): sum(bm^2) = 4 pi sum_{k,k'} w_k w_k' e^{-(dy^2+dx^2)/16},
    a handful of tiny [100,100] ops.  The hard 9x9 window of the
    reference changes the loss by ~1e-5 relative and is dropped.
  * Coordinate broadcasts come straight from DRAM: one [128, 2, 100]
    partition-stride-0 DMA per image supplies the transposed-factor
    input AND the pairwise dx/dy matrices - no on-chip broadcasts on
    the critical path (SWDGE broadcasts queue behind the prediction
    stream on the shared DMA semaphore lanes; PSUM matmul broadcasts
    run half-rate cold and stall the PE queue).
  * s matmuls run in float32r (full PE rate at moving free dim 512).
  * Sharding: data-parallel over batch, 2 images per core, 8 cores.
    Prediction tiles stream on the sync HWDGE ring from instruction 0;
    coordinates ride the scalar HWDGE ring.
"""

import numpy as np

import concourse.bass as bass
import concourse.bass_isa as bass_isa
import concourse.bacc as bacc
import concourse.mybir as mybir
from concourse import tile
from concourse.bass_utils import run_bass_kernel_spmd

F32 = mybir.dt.float32
F32R = mybir.dt.float32r
I32 = mybir.dt.int32
OP = mybir.AluOpType
AF = mybir.ActivationFunctionType
AX = mybir.AxisListType

B, H, W = 16, 1024, 1024
NKP = 100
NCORES = 8
IMGS = B // NCORES            # 2 images per core
ROWBLK = 2                    # 128-row blocks per DMA tile
NBLK = H // 128               # 8 row blocks per image
NCHUNK = NBLK // ROWBLK       # 4 tiles per image
NT = IMGS * NCHUNK            # 8 tiles per core
TW = NBLK * NKP               # 800: transposed y-factor free width

# square-accumulate engine per tile; last tile split ACT/DVE for the tail
SQ_ENGINE = ["act", "bn", "act", "act", "bn", "act", "act", "split"]

# out columns: 0..7 ACT square-accum partials, 9..10 cross terms per image
# (partitions 0:100), 11..12 bm^2 per image, 14..23 bn_stats (mean, var)
# pairs for the DVE-side p^2 tiles (host converts to sum-of-squares)
NOUT = 24
BN_TILES = [t for t, e in enumerate(SQ_ENGINE) if e != "act"]


def build_nc():
    nc = bacc.Bacc(None, target_bir_lowering=False)

    pred = nc.dram_tensor("pred", [IMGS, H, W], F32, kind="ExternalInput")
    coords = nc.dram_tensor("coords", [IMGS, NKP, 2], I32, kind="ExternalInput")
    ltri_c = nc.dram_tensor("ltri_c", [NKP, 2 * NKP], F32,
                            kind="ExternalInput")
    out = nc.dram_tensor("partial", [128, NOUT], F32, kind="ExternalOutput")

    with tile.TileContext(nc) as tc:
        with (
            tc.tile_pool(name="const", bufs=1) as constp,
            tc.tile_pool(name="pred", bufs=NT) as predp,
            tc.tile_pool(name="fact", bufs=2) as factp,
            tc.tile_pool(name="small", bufs=2) as smallp,
            tc.tile_pool(name="junk", bufs=3) as junkp,
            tc.tile_pool(name="psum", bufs=1, space="PSUM") as psump,
        ):
            # ---- prediction stream: issue all 8 x 1MB DMAs immediately ----
            pred_v = pred.rearrange("i (c b p) w -> i c p b w", b=ROWBLK, p=128)
            ptiles = []
            for i in range(IMGS):
                for c in range(NCHUNK):
                    pt = predp.tile([128, ROWBLK, W], F32R, tag="pt")
                    nc.sync.dma_start(pt[:], pred_v[i, c].bitcast(F32R))
                    ptiles.append(pt)

            # ---- coordinate broadcasts from DRAM on the scalar ring:
            # cbb[i][p, t, k] = coords[i, k, t] for every partition p ----
            cb2 = constp.tile([128, IMGS, 2 * NKP], I32)
            for i in range(IMGS):
                flat = coords[i].rearrange("n t -> (n t)").unsqueeze(0)
                nc.scalar.dma_start(cb2[:, i, :],
                                    flat.broadcast_to([128, 2 * NKP]))
            cc4i = constp.tile([NKP, 2 * IMGS], I32)
            for i in range(IMGS):
                nc.scalar.dma_start(cc4i[:, 2 * i:2 * i + 2], coords[i])
            ltri2x = constp.tile([NKP, 2 * NKP], F32)
            nc.scalar.dma_start(ltri2x[:], ltri_c[:])

            # ---- on-chip constants ----
            riota = constp.tile([128, IMGS, NBLK, NKP], F32)  # r + 128b
            nc.gpsimd.iota(riota[:], [[0, IMGS], [128, NBLK], [0, NKP]],
                           channel_multiplier=1,
                           allow_small_or_imprecise_dtypes=True)
            iota_row = constp.tile([128, W], F32)
            nc.gpsimd.iota(iota_row[:], [[1, W]], channel_multiplier=0,
                           allow_small_or_imprecise_dtypes=True)
            accbig = constp.tile([128, NOUT], F32)
            nc.vector.memset(accbig[:], 0.0)

            # ---- critical chain first: transposed y factors, both images
            # in one pass: fTb[r, (i,b,k)] = u(r + 128b - y_k^i) ----
            cbf2 = constp.tile([128, IMGS, 2 * NKP], F32)
            nc.vector.tensor_copy(cbf2[:], cb2[:])
            dTb = factp.tile([128, IMGS, NBLK, NKP], F32, tag="dTb", bufs=1)
            ybx = cbf2[:].rearrange("p i (n t) -> p i n t",
                                    t=2)[:, :, :, 1].unsqueeze(2)
            nc.vector.tensor_tensor(dTb[:], riota[:],
                                    ybx.broadcast_to([128, IMGS, NBLK, NKP]),
                                    OP.subtract)
            dsqTb = factp.tile([128, IMGS * TW], F32, tag="dsqTb", bufs=1)
            nc.scalar.activation(dsqTb[:],
                                 dTb[:].rearrange("p i b n -> p (i b n)"),
                                 AF.Square)
            fTb = factp.tile([128, IMGS * TW], F32R, tag="fTb", bufs=1)
            nc.scalar.activation(fTb[:], dsqTb[:], AF.Exp, scale=-0.125)
            fT = [fTb[:, 0:TW], fTb[:, TW:2 * TW]]

            cc4f = constp.tile([NKP, 2 * IMGS], F32)
            nc.vector.tensor_copy(cc4f[:], cc4i[:])

            xf, s_ps, wcs = [], [], []
            for i in range(IMGS):
                xcol = cc4f[:, 2 * i:2 * i + 1]
                ycol = cc4f[:, 2 * i + 1:2 * i + 2]
                cbv = cbf2[:].rearrange("p i (n t) -> p i n t", t=2)
                xbb = cbv[0:NKP, i, :, 0]   # [100, 100]: x_j everywhere
                ybb = cbv[0:NKP, i, :, 1]   # [100, 100]: y_j everywhere

                # ---- row x factors (dedup weight applied post-reduce):
                # xf[k, c] = u(c - x_k) ----
                dsqX = factp.tile([NKP, W], F32, tag="dsqX")
                nc.scalar.activation(dsqX[:], iota_row[0:NKP, :], AF.Square,
                                     bias=xcol, scale=-1.0)
                x = factp.tile([NKP, W], F32, tag=f"xf{i}", bufs=1)
                nc.scalar.activation(x[:], dsqX[:], AF.Exp, scale=-0.125)
                xf.append(x)

                # ---- sum(bm^2) via the Gaussian-sum identity, plus
                # dedup weights - small DVE ops with host-DMA'd triangle
                # masks; one gpsimd all-reduce for the row broadcast ----
                dxm = smallp.tile([NKP, NKP], F32, tag="dxm")
                nc.vector.tensor_scalar(dxm[:], xbb, xcol, None, OP.subtract)
                dym = smallp.tile([NKP, NKP], F32, tag="dym")
                nc.vector.tensor_scalar(dym[:], ybb, ycol, None, OP.subtract)
                nc.vector.tensor_tensor(dxm[:], dxm[:], dxm[:], OP.mult)
                nc.vector.tensor_tensor(dym[:], dym[:], dym[:], OP.mult)
                dsm = smallp.tile([NKP, NKP], F32, tag="dsm")
                nc.vector.tensor_tensor(dsm[:], dxm[:], dym[:], OP.add)
                # exact-duplicate test: dsm == 0
                eq = smallp.tile([NKP, NKP], F32, tag="eq")
                nc.vector.tensor_scalar(eq[:], dsm[:], 0.0, None, OP.is_equal)
                # dupc[p] = # earlier duplicates of p (cols j < p)
                ejunk2 = smallp.tile([NKP, NKP], F32, tag="ejunk2")
                nc.vector.tensor_tensor(ejunk2[:], eq[:],
                                        ltri2x[:, NKP:2 * NKP], OP.mult)
                dupc = smallp.tile([NKP, 1], F32, tag="dupc")
                nc.vector.tensor_reduce(dupc[:], ejunk2[:], axis=AX.X,
                                        op=OP.add)
                wc = smallp.tile([NKP, 1], F32, tag="wc")
                nc.vector.tensor_scalar(wc[:], dupc[:], 0.0, None,
                                        OP.is_equal)
                wcs.append(wc)
                # row-side weight, broadcast: wrb[p, j] = w_j
                ejunk = smallp.tile([NKP, NKP], F32, tag="ejunk")
                nc.vector.tensor_tensor(ejunk[:], eq[:], ltri2x[:, 0:NKP],
                                        OP.mult)
                duprb = smallp.tile([NKP, NKP], F32, tag="duprb")
                nc.gpsimd.partition_all_reduce(duprb[:], ejunk[:],
                                               channels=NKP,
                                               reduce_op=bass_isa.ReduceOp.add)
                wrb = smallp.tile([NKP, NKP], F32, tag="wrb")
                nc.vector.tensor_scalar(wrb[:], duprb[:], 0.0, None,
                                        OP.is_equal)
                eg = smallp.tile([NKP, NKP], F32, tag="eg")
                nc.scalar.activation(eg[:], dsm[:], AF.Exp, scale=-0.0625)
                nc.vector.tensor_tensor(eg[:], eg[:], wrb[:], OP.mult)
                egc = smallp.tile([NKP, 1], F32, tag="egc")
                nc.vector.tensor_reduce(egc[:], eg[:], axis=AX.X, op=OP.add)
                nc.vector.tensor_scalar(accbig[0:NKP, 11 + i:12 + i],
                                        egc[:], wc[:], None, OP.mult)

                sp = psump.tile([NKP, W], F32, tag=f"s{i}", name=f"s{i}")
                s_ps.append(sp)

            # ---- per-tile: s matmuls + p^2 square-accumulate ----
            for t, pt in enumerate(ptiles):
                i, c = t // NCHUNK, t % NCHUNK
                for b2 in range(ROWBLK):
                    blk = ROWBLK * c + b2
                    for h in range(2):
                        nc.tensor.matmul(
                            s_ps[i][:, 512 * h:512 * (h + 1)],
                            fT[i][:, NKP * blk:NKP * (blk + 1)],
                            pt[:, b2, 512 * h:512 * (h + 1)],
                            start=(blk == 0), stop=(blk == NBLK - 1))

                eng = SQ_ENGINE[t]
                pv = pt[:].bitcast(F32)
                pv4 = pv.rearrange("p b (u w) -> p (b u) w", u=2)
                bcol = 14 + 2 * BN_TILES.index(t) if eng != "act" else None
                if eng == "act":
                    junk = junkp.tile([128, ROWBLK, W], F32, tag="junksq")
                    nc.scalar.activation(junk[:], pv, AF.Square,
                                         accum_out=accbig[:, t:t + 1])
                elif eng == "bn":
                    bno = junkp.tile([128, 4, 6], F32, tag="bno", bufs=2)
                    for u in range(4):
                        nc.vector.bn_stats(bno[:, u, :], pv4[:, u, :])
                    nc.vector.bn_aggr(accbig[:, bcol:bcol + 2], bno[:])
                else:  # split halves across ACT and DVE for a short tail
                    junk = junkp.tile([128, ROWBLK, W], F32, tag="junksq")
                    nc.scalar.activation(junk[:, 0], pv[:, 0], AF.Square,
                                         accum_out=accbig[:, t:t + 1])
                    bno = junkp.tile([128, 2, 6], F32, tag="bno2", bufs=2)
                    for u in range(2):
                        nc.vector.bn_stats(bno[:, u, :], pv4[:, 2 + u, :])
                    nc.vector.bn_aggr(accbig[:, bcol:bcol + 2], bno[:])

                # cross term as soon as this image's s accumulation closes
                if c == NCHUNK - 1:
                    junkx = junkp.tile([NKP, W], F32, tag="junkx", bufs=2)
                    nc.vector.tensor_tensor(junkx[:], s_ps[i][:], xf[i][:],
                                            OP.mult)
                    crx = junkp.tile([NKP, 1], F32, tag="crx", bufs=2)
                    nc.vector.tensor_reduce(crx[:], junkx[:], axis=AX.X,
                                            op=OP.add)
                    nc.vector.tensor_scalar(accbig[0:NKP, 9 + i:10 + i],
                                            crx[:], wcs[i][:], None, OP.mult)

            nc.sync.dma_start(out[:], accbig[:])

    nc.compile()
    return nc


_NC_CACHE = {}


def _get_nc():
    if "nc" not in _NC_CACHE:
        _NC_CACHE["nc"] = build_nc()
    return _NC_CACHE["nc"]


def _run(prediction, coordinates, **kw):
    nc = _get_nc()
    pred = np.ascontiguousarray(np.asarray(prediction), dtype=np.float32)
    crds = np.ascontiguousarray(np.asarray(coordinates), dtype=np.int32)
    assert pred.shape == (B, 1, H, W) and crds.shape == (B, NKP, 2)
    jjj = np.arange(NKP)
    ltri = (jjj[None, :] > jjj[:, None]).astype(np.float32)
    ltri2 = (jjj[None, :] < jjj[:, None]).astype(np.float32)
    ltri_c = np.ascontiguousarray(np.concatenate([ltri, ltri2], axis=1))
    in_maps = []
    for core in range(NCORES):
        sl = slice(core * IMGS, (core + 1) * IMGS)
        in_maps.append({
            "pred": np.ascontiguousarray(pred[sl, 0]),
            "coords": np.ascontiguousarray(crds[sl]),
            "ltri_c": ltri_c,
        })
    res = run_bass_kernel_spmd(nc, in_maps, core_ids=list(range(NCORES)), **kw)
    total = 0.0
    for r in res.results:
        p = r["partial"].astype(np.float64)
        sq = p[:, 0:8].sum()
        for j, t in enumerate(BN_TILES):
            n = ROWBLK * W if SQ_ENGINE[t] == "bn" else W
            mean = p[:, 14 + 2 * j]
            var = p[:, 15 + 2 * j]
            sq += (n * (var + mean ** 2)).sum()
        cross = p[0:NKP, 9:11].sum()
        bm2 = 4.0 * np.pi * p[0:NKP, 11:13].sum()
        total += sq - 2.0 * cross + bm2
    loss = np.asarray(total / (B * H * W), dtype=np.float32)
    return loss, res


def kernel(prediction, coordinates, labels=None, gaussian_kernel=None, **kw):
    loss, _ = _run(prediction, coordinates)
    return loss


# revision 27
# speedup vs baseline: 1.0273x; 1.0273x over previous
"""Trainium2 Bass kernel for nn_LossWithBeliveMaps.

loss = mean((prediction - belive_map)^2) where belive_map is 100 Gaussian
(9x9, sigma=2) stamps per image, scattered at integer keypoint coordinates.

v4 design: never materialize the dense believe map.  Expand the MSE:

    sum((p - bm)^2) = sum(p^2) - 2*sum(p*bm) + sum(bm^2)

  * sum(p^2): square+row-accumulate each prediction tile as it lands,
    split between ScalarE (activation accum_out) and DVE (bn_stats; the
    host converts mean/var back to a sum of squares).
  * The Gaussian is separable/rank-1: u(d) = exp(-d^2/8), so with
    yfT[r, k] = u(r - y_k) (r on partitions) a K-accumulating TensorE
    matmul computes s[k, c] = sum_r yfT[r, k] * p[r, c] and
        sum(p*bm) = sum_{k,c} s[k,c] * w_k * u(c - x_k)
    where the dedup weight w_k rides in the xf row factors (exp bias).
  * sum(bm^2) uses the Gaussian-sum identity
    sum_r u(r-y)u(r-y') = sqrt(4 pi) exp(-(y-y')^2/16) (exact for interior
    keypoints): sum(bm^2) = 4 pi sum_{k,k'} w_k w_k' e^{-(dy^2+dx^2)/16},
    a handful of tiny [100,100] ops.  The hard 9x9 window of the
    reference changes the loss by ~1e-5 relative and is dropped.
  * Coordinate broadcasts come straight from DRAM: one [128, 2, 100]
    partition-stride-0 DMA per image supplies the transposed-factor
    input AND the pairwise dx/dy matrices - no on-chip broadcasts on
    the critical path (SWDGE broadcasts queue behind the prediction
    stream on the shared DMA semaphore lanes; PSUM matmul broadcasts
    run half-rate cold and stall the PE queue).
  * s matmuls run in float32r (full PE rate at moving free dim 512).
  * Sharding: data-parallel over batch, 2 images per core, 8 cores.
    Prediction tiles stream on the sync HWDGE ring from instruction 0;
    coordinates ride the scalar HWDGE ring.
"""

import numpy as np

import concourse.bass as bass
import concourse.bass_isa as bass_isa
import concourse.bacc as bacc
import concourse.mybir as mybir
from concourse import tile
from concourse.bass_utils import run_bass_kernel_spmd

F32 = mybir.dt.float32
F32R = mybir.dt.float32r
I32 = mybir.dt.int32
OP = mybir.AluOpType
AF = mybir.ActivationFunctionType
AX = mybir.AxisListType

B, H, W = 16, 1024, 1024
NKP = 100
NCORES = 8
IMGS = B // NCORES            # 2 images per core
ROWBLK = 2                    # 128-row blocks per DMA tile
NBLK = H // 128               # 8 row blocks per image
NCHUNK = NBLK // ROWBLK       # 4 tiles per image
NT = IMGS * NCHUNK            # 8 tiles per core
TW = NBLK * NKP               # 800: transposed y-factor free width

# square-accumulate engine per tile; last tile split ACT/DVE for the tail
SQ_ENGINE = ["act", "bn", "act", "act", "bn", "act", "act", "act", "bn"]

# out columns: 0..7 ACT square-accum partials, 9..10 cross terms per image
# (partitions 0:100), 11..12 bm^2 per image, 14..23 bn_stats (mean, var)
# pairs for the DVE-side p^2 tiles (host converts to sum-of-squares)
NOUT = 24
BN_TILES = [t for t, e in enumerate(SQ_ENGINE) if e != "act"]


def build_nc():
    nc = bacc.Bacc(None, target_bir_lowering=False)

    pred = nc.dram_tensor("pred", [IMGS, H, W], F32, kind="ExternalInput")
    coords = nc.dram_tensor("coords", [IMGS, NKP, 2], I32, kind="ExternalInput")
    ltri_c = nc.dram_tensor("ltri_c", [NKP, 2 * NKP], F32,
                            kind="ExternalInput")
    out = nc.dram_tensor("partial", [128, NOUT], F32, kind="ExternalOutput")

    with tile.TileContext(nc) as tc:
        with (
            tc.tile_pool(name="const", bufs=1) as constp,
            tc.tile_pool(name="pred", bufs=NT) as predp,
            tc.tile_pool(name="fact", bufs=2) as factp,
            tc.tile_pool(name="small", bufs=2) as smallp,
            tc.tile_pool(name="junk", bufs=3) as junkp,
            tc.tile_pool(name="psum", bufs=1, space="PSUM") as psump,
        ):
            # ---- prediction stream: issue all 8 x 1MB DMAs immediately ----
            pred_v = pred.rearrange("i (c b p) w -> i c p b w", b=ROWBLK, p=128)
            ptiles = []
            for i in range(IMGS):
                for c in range(NCHUNK):
                    if i == IMGS - 1 and c == NCHUNK - 1:
                        for b2 in range(ROWBLK):
                            ph = predp.tile([128, 1, W], F32R, tag="ph",
                                            name=f"ph{b2}", bufs=2)
                            nc.sync.dma_start(
                                ph[:], pred_v[i, c][:, b2:b2 + 1, :].bitcast(F32R))
                            ptiles.append(ph)
                    else:
                        pt = predp.tile([128, ROWBLK, W], F32R, tag="pt")
                        nc.sync.dma_start(pt[:], pred_v[i, c].bitcast(F32R))
                        ptiles.append(pt)

            # ---- coordinate broadcasts from DRAM on the scalar ring:
            # cbb[i][p, t, k] = coords[i, k, t] for every partition p ----
            cb2 = constp.tile([128, IMGS, 2 * NKP], I32)
            for i in range(IMGS):
                flat = coords[i].rearrange("n t -> (n t)").unsqueeze(0)
                nc.scalar.dma_start(cb2[:, i, :],
                                    flat.broadcast_to([128, 2 * NKP]))
            cc4i = constp.tile([NKP, 2 * IMGS], I32)
            for i in range(IMGS):
                nc.scalar.dma_start(cc4i[:, 2 * i:2 * i + 2], coords[i])
            ltri2x = constp.tile([NKP, 2 * NKP], F32)
            nc.scalar.dma_start(ltri2x[:], ltri_c[:])

            # ---- on-chip constants ----
            riota = constp.tile([128, IMGS, NBLK, NKP], F32)  # r + 128b
            nc.gpsimd.iota(riota[:], [[0, IMGS], [128, NBLK], [0, NKP]],
                           channel_multiplier=1,
                           allow_small_or_imprecise_dtypes=True)
            iota_row = constp.tile([128, W], F32)
            nc.gpsimd.iota(iota_row[:], [[1, W]], channel_multiplier=0,
                           allow_small_or_imprecise_dtypes=True)
            accbig = constp.tile([128, NOUT], F32)
            nc.vector.memset(accbig[:], 0.0)

            # ---- critical chain first: transposed y factors, both images
            # in one pass: fTb[r, (i,b,k)] = u(r + 128b - y_k^i) ----
            cbf2 = constp.tile([128, IMGS, 2 * NKP], F32)
            nc.vector.tensor_copy(cbf2[:], cb2[:])
            dTb = factp.tile([128, IMGS, NBLK, NKP], F32, tag="dTb", bufs=1)
            ybx = cbf2[:].rearrange("p i (n t) -> p i n t",
                                    t=2)[:, :, :, 1].unsqueeze(2)
            nc.vector.tensor_tensor(dTb[:], riota[:],
                                    ybx.broadcast_to([128, IMGS, NBLK, NKP]),
                                    OP.subtract)
            dsqTb = factp.tile([128, IMGS * TW], F32, tag="dsqTb", bufs=1)
            nc.scalar.activation(dsqTb[:],
                                 dTb[:].rearrange("p i b n -> p (i b n)"),
                                 AF.Square)
            fTb = factp.tile([128, IMGS * TW], F32R, tag="fTb", bufs=1)
            nc.scalar.activation(fTb[:], dsqTb[:], AF.Exp, scale=-0.125)
            fT = [fTb[:, 0:TW], fTb[:, TW:2 * TW]]

            cc4f = constp.tile([NKP, 2 * IMGS], F32)
            nc.vector.tensor_copy(cc4f[:], cc4i[:])

            xf, s_ps, wcs = [], [], []
            for i in range(IMGS):
                xcol = cc4f[:, 2 * i:2 * i + 1]
                ycol = cc4f[:, 2 * i + 1:2 * i + 2]
                cbv = cbf2[:].rearrange("p i (n t) -> p i n t", t=2)
                xbb = cbv[0:NKP, i, :, 0]   # [100, 100]: x_j everywhere
                ybb = cbv[0:NKP, i, :, 1]   # [100, 100]: y_j everywhere

                # ---- row x factors (dedup weight applied post-reduce):
                # xf[k, c] = u(c - x_k) ----
                dsqX = factp.tile([NKP, W], F32, tag="dsqX")
                nc.scalar.activation(dsqX[:], iota_row[0:NKP, :], AF.Square,
                                     bias=xcol, scale=-1.0)
                x = factp.tile([NKP, W], F32, tag=f"xf{i}", bufs=1)
                nc.scalar.activation(x[:], dsqX[:], AF.Exp, scale=-0.125)
                xf.append(x)

                # ---- sum(bm^2) via the Gaussian-sum identity, plus
                # dedup weights - small DVE ops with host-DMA'd triangle
                # masks; one gpsimd all-reduce for the row broadcast ----
                dxm = smallp.tile([NKP, NKP], F32, tag="dxm")
                nc.vector.tensor_scalar(dxm[:], xbb, xcol, None, OP.subtract)
                dym = smallp.tile([NKP, NKP], F32, tag="dym")
                nc.vector.tensor_scalar(dym[:], ybb, ycol, None, OP.subtract)
                nc.vector.tensor_tensor(dxm[:], dxm[:], dxm[:], OP.mult)
                nc.vector.tensor_tensor(dym[:], dym[:], dym[:], OP.mult)
                dsm = smallp.tile([NKP, NKP], F32, tag="dsm")
                nc.vector.tensor_tensor(dsm[:], dxm[:], dym[:], OP.add)
                # exact-duplicate test: dsm == 0
                eq = smallp.tile([NKP, NKP], F32, tag="eq")
                nc.vector.tensor_scalar(eq[:], dsm[:], 0.0, None, OP.is_equal)
                # dupc[p] = # earlier duplicates of p (cols j < p)
                ejunk2 = smallp.tile([NKP, NKP], F32, tag="ejunk2")
                nc.vector.tensor_tensor(ejunk2[:], eq[:],
                                        ltri2x[:, NKP:2 * NKP], OP.mult)
                dupc = smallp.tile([NKP, 1], F32, tag="dupc")
                nc.vector.tensor_reduce(dupc[:], ejunk2[:], axis=AX.X,
                                        op=OP.add)
                wc = smallp.tile([NKP, 1], F32, tag="wc")
                nc.vector.tensor_scalar(wc[:], dupc[:], 0.0, None,
                                        OP.is_equal)
                wcs.append(wc)
                # row-side weight, broadcast: wrb[p, j] = w_j
                ejunk = smallp.tile([NKP, NKP], F32, tag="ejunk")
                nc.vector.tensor_tensor(ejunk[:], eq[:], ltri2x[:, 0:NKP],
                                        OP.mult)
                duprb = smallp.tile([NKP, NKP], F32, tag="duprb")
                nc.gpsimd.partition_all_reduce(duprb[:], ejunk[:],
                                               channels=NKP,
                                               reduce_op=bass_isa.ReduceOp.add)
                wrb = smallp.tile([NKP, NKP], F32, tag="wrb")
                nc.vector.tensor_scalar(wrb[:], duprb[:], 0.0, None,
                                        OP.is_equal)
                eg = smallp.tile([NKP, NKP], F32, tag="eg")
                nc.scalar.activation(eg[:], dsm[:], AF.Exp, scale=-0.0625)
                nc.vector.tensor_tensor(eg[:], eg[:], wrb[:], OP.mult)
                egc = smallp.tile([NKP, 1], F32, tag="egc")
                nc.vector.tensor_reduce(egc[:], eg[:], axis=AX.X, op=OP.add)
                nc.vector.tensor_scalar(accbig[0:NKP, 11 + i:12 + i],
                                        egc[:], wc[:], None, OP.mult)

                sp = psump.tile([NKP, W], F32, tag=f"s{i}", name=f"s{i}")
                s_ps.append(sp)

            # ---- per-tile: s matmuls + p^2 square-accumulate.
            # blocks[t] = (image, [global block indices], ROWBLK dim) ----
            blocks = []
            for i in range(IMGS):
                for c in range(NCHUNK):
                    if i == IMGS - 1 and c == NCHUNK - 1:
                        blocks.append((i, [ROWBLK * c], 1))
                        blocks.append((i, [ROWBLK * c + 1], 1))
                    else:
                        blocks.append((i, [ROWBLK * c + b2
                                           for b2 in range(ROWBLK)], ROWBLK))

            def sq_act(pv, col):
                junk = junkp.tile([128, pv.shape[1], W], F32, tag="junksq",
                                  name="junk")
                nc.scalar.activation(junk[:], pv, AF.Square,
                                     accum_out=accbig[:, col:col + 1])

            def sq_bn(pv4, nbn, bcol):
                bno = junkp.tile([128, nbn, 6], F32, tag=f"bno{nbn}",
                                 name="bno", bufs=2)
                for u in range(nbn):
                    nc.vector.bn_stats(bno[:, u, :], pv4[:, u, :])
                nc.vector.bn_aggr(accbig[:, bcol:bcol + 2], bno[:])

            def cross(i):
                junkx = junkp.tile([NKP, W], F32, tag="junkx", bufs=2)
                nc.vector.tensor_tensor(junkx[:], s_ps[i][:], xf[i][:],
                                        OP.mult)
                crx = junkp.tile([NKP, 1], F32, tag="crx", bufs=2)
                nc.scalar.activation(junkx[:], junkx[:], AF.Copy,
                                     accum_out=crx[:])
                nc.vector.tensor_scalar(accbig[0:NKP, 9 + i:10 + i],
                                        crx[:], wcs[i][:], None, OP.mult)

            for t, pt in enumerate(ptiles):
                i, blks, rb = blocks[t]
                for b2, blk in enumerate(blks):
                    for h in range(2):
                        nc.tensor.matmul(
                            s_ps[i][:, 512 * h:512 * (h + 1)],
                            fT[i][:, NKP * blk:NKP * (blk + 1)],
                            pt[:, b2, 512 * h:512 * (h + 1)],
                            start=(blk == 0), stop=(blk == NBLK - 1))
                last = (blks[-1] == NBLK - 1)
                if last:
                    cross(i)  # emit ahead of the final bn for a short tail

                eng = SQ_ENGINE[t]
                pv = pt[:].bitcast(F32)
                pv4 = pv.rearrange("p b (u w) -> p (b u) w", u=2)
                if eng == "act":
                    sq_act(pv, t)
                else:
                    sq_bn(pv4, 2 * rb, 14 + 2 * BN_TILES.index(t))

            nc.sync.dma_start(out[:], accbig[:])

    nc.compile()
    return nc


_NC_CACHE = {}


def _get_nc():
    if "nc" not in _NC_CACHE:
        _NC_CACHE["nc"] = build_nc()
    return _NC_CACHE["nc"]


def _run(prediction, coordinates, **kw):
    nc = _get_nc()
    pred = np.ascontiguousarray(np.asarray(prediction), dtype=np.float32)
    crds = np.ascontiguousarray(np.asarray(coordinates), dtype=np.int32)
    assert pred.shape == (B, 1, H, W) and crds.shape == (B, NKP, 2)
    jjj = np.arange(NKP)
    ltri = (jjj[None, :] > jjj[:, None]).astype(np.float32)
    ltri2 = (jjj[None, :] < jjj[:, None]).astype(np.float32)
    ltri_c = np.ascontiguousarray(np.concatenate([ltri, ltri2], axis=1))
    in_maps = []
    for core in range(NCORES):
        sl = slice(core * IMGS, (core + 1) * IMGS)
        in_maps.append({
            "pred": np.ascontiguousarray(pred[sl, 0]),
            "coords": np.ascontiguousarray(crds[sl]),
            "ltri_c": ltri_c,
        })
    res = run_bass_kernel_spmd(nc, in_maps, core_ids=list(range(NCORES)), **kw)
    total = 0.0
    for r in res.results:
        p = r["partial"].astype(np.float64)
        sq = p[:, 0:9].sum()
        for j, t in enumerate(BN_TILES):
            n = W if t >= 7 else ROWBLK * W
            mean = p[:, 14 + 2 * j]
            var = p[:, 15 + 2 * j]
            sq += (n * (var + mean ** 2)).sum()
        cross = p[0:NKP, 9:11].sum()
        bm2 = 4.0 * np.pi * p[0:NKP, 11:13].sum()
        total += sq - 2.0 * cross + bm2
    loss = np.asarray(total / (B * H * W), dtype=np.float32)
    return loss, res


def kernel(prediction, coordinates, labels=None, gaussian_kernel=None, **kw):
    loss, _ = _run(prediction, coordinates)
    return loss
